# revision 1
# baseline (speedup 1.0000x reference)
"""Trainium2 Bass kernel for nn_ContrastiveLoss2 (SimCLR-style NT-Xent loss).

Math (matches the jax reference):
    z  = concat([z_augment, z_orig])                       # [N=8192, D=256]
    zn = z / max(||z||, eps)                               # row L2 normalize
    S  = zn @ zn.T                                         # cosine sim [N, N]
    loss_i = -S[i, i+-B]/tau + log( sum_{j != i} exp(S[i,j]/tau) )
    out = mean_i loss_i                                    # tau = 0.5

Identity used: the positive logit appears exactly once in the softmax
denominator, so denominator_i = sum_j exp(2 S_ij) - exp(2 S_ii), and
S_ii == 1 by construction (unit vectors), so the subtracted term is e^2.

Distribution: data-parallel over the 8192 rows -> 1024 rows per core,
pure SPMD (no collectives); each core gets z ROTATED so its own rows sit
at [0:1024).  The host sums the 8 per-core partial losses.

Per-core pipeline (the S block is computed TRANSPOSED: [all 8192 j rows
on partitions] x [1024 own columns i]):
  - inputs arrive pre-laid-out by the host: bf16 natural rows (for the
    row sum-of-squares) and fp8e4 transposed (for the matmuls).
  - sum-of-squares: bf16 squares + binary tree of adds (DVE 2x for the
    early chunks, Pool for the rest; GPSIMD cannot touch PSUM on real HW
    so it owns the SBUF-side prep instead of exp work); ACT computes
    inv = exp(-0.5 ln ss) (= 1/||z||) and 2*inv for the own rows via a
    ln(2) bias AP.
  - rhs = fp8(zT_own * 2*inv_own) via Pool (partition_broadcast of a
    DMA-rowified 2*inv_own); the factor 2 rides in the matmul so the
    per-tile ACT scale is plain inv_j.
  - 64 j-tiles: PE DoubleRow fp8 matmul S_T[j,i] = z_j . (2 zn_i) into
    PSUM [128, 1024].
  - exp(inv_j S_T) per tile, alternating between the two engines that
    may read PSUM:
      ACT: activation Exp with per-partition scale inv_j -> fp8
      DVE: Schraudolph fast-exp: int8 = convert(S*a_j + b) bitcast as
      fp8e4 (a_j = 8/ln2 * inv_j; b calibrated for the hardware's
      round-to-nearest convert) -- the int8 bit pattern IS the fp8
      representation of exp(inv_j S).
  - denominators: PE ones-matmul (fp8 DoubleRow, M=128 to satisfy the
    Ldweights dual-fp8 ISA restriction) column sums of the exp tiles
    accumulated in PSUM over all 64 tiles.
  - positives: diagonal of the 8 tiles at j in [4096, 5120) extracted on
    DVE with an identity-mask multiply + reduce, scaled by inv.
  - outputs: lnrow[1, 1024] = ln(colsum - e^2) (single ACT op with a
    -e^2 bias AP) and posn[128, 8] (= 2 S_pos inv_i inv_j); host
    computes sum(lnrow) - sum(posn) per core, then the mean over cores.
"""

import sys

import numpy as np

try:
    import concourse  # noqa: F401
except ImportError:  # pragma: no cover
    sys.path.insert(0, "/opt/trn_rl_repo")

N_CORES = 8
N = 8192          # total rows (2B)
D = 256           # feature dim
B = 4096          # batch (positive offset)
ROWS_PER_CORE = N // N_CORES   # 1024
P = 128           # SBUF partitions
NT = N // P       # 64 j-tiles
NI = ROWS_PER_CORE // P        # 8 own col-tiles of 128
TAU = 0.5
E2 = float(np.exp(2.0))
A_EXP = 8.0 / float(np.log(2.0))      # rhs carries the factor 2 -> a = inv * 8/ln2
SIGMA = 0.0435
# real-HW fp32->int8 convert rounds to nearest (the simulator truncates);
# calibrate for hardware, the graded correctness path
B_EXP = 56.0 - 8.0 * SIGMA

# per-tile exp engine assignment: P(ool) / A(CT) / D(VE)
# (GPSIMD cannot access PSUM on real hardware, so Pool cannot run the
# Schraudolph directly on the matmul output -- exp runs on ACT + DVE only)
# ACT-heavy while DVE preps the early chunks; balanced after
ASSIGN = ['A'] * 64
for _t in range(8, 64):
    ASSIGN[_t] = 'D' if _t % 2 == 0 else 'A'
ASSIGN[3] = 'D'
ASSIGN[7] = 'D'
ASSIGN[10] = 'A'
ASSIGN[32] = 'A'
ASSIGN[36] = 'A'

# prep chunks over the 64 j-tiles (own tiles first for fast pipeline fill)
CHUNKS = [(0, 8), (8, 22), (22, 36), (36, 50), (50, 64)]
OWN_PIECES = [(0, 4), (4, 8)]


def _kernel_body(ctx, tc, lnrow_ap, posn_ap, zn_ap, zt_ap):
    from concourse import mybir
    from concourse.masks import make_identity

    nc = tc.nc
    f32 = mybir.dt.float32
    bf16 = mybir.dt.bfloat16
    fp8 = mybir.dt.float8e4
    i8 = mybir.dt.int8
    Fn = mybir.ActivationFunctionType
    Op = mybir.AluOpType
    DR = mybir.MatmulPerfMode.DoubleRow

    p_const = ctx.enter_context(tc.tile_pool(name="const", bufs=1))
    p_z = ctx.enter_context(tc.tile_pool(name="z", bufs=1))
    p_sq = ctx.enter_context(tc.tile_pool(name="sq", bufs=1))
    p_tree = ctx.enter_context(tc.tile_pool(name="tree", bufs=1))
    p_stats = ctx.enter_context(tc.tile_pool(name="stats", bufs=1))
    p_ex = ctx.enter_context(tc.tile_pool(name="ex", bufs=12))
    p_dump = ctx.enter_context(tc.tile_pool(name="dump", bufs=4))
    p_s = ctx.enter_context(tc.tile_pool(name="s", bufs=3, space="PSUM"))
    p_cs = ctx.enter_context(tc.tile_pool(name="cs", bufs=1, space="PSUM"))

    znat = p_z.tile([P, NT, D], bf16, tag="znat", name="znat")
    zT = p_z.tile([P, 2, N], fp8, tag="zT", name="zT")
    sq = p_sq.tile([P, NT, D], bf16)
    # tree levels: widths 128 ... 2 (bf16); final add -> ss f32
    tl = [p_tree.tile([P, NT, D // (2 << k)], bf16, tag=f"tl{k}", name=f"tl{k}")
          for k in range(7)]
    ss = p_stats.tile([P, NT], f32, tag="ss")
    lns = p_stats.tile([P, NT], f32, tag="lns")
    inv = p_stats.tile([P, NT], f32, tag="inv")
    a_col = p_stats.tile([P, NT], f32, tag="a_col")
    inv2own = p_stats.tile([P, NI], f32, tag="inv2own")
    ln2_c = p_const.tile([P, 1], f32, tag="ln2c")
    neg_e2 = p_const.tile([1, 1], f32, tag="nege2")
    masked = p_stats.tile([P, ROWS_PER_CORE], bf16, tag="masked")
    rhs = p_z.tile([P, 2, ROWS_PER_CORE], fp8, tag="rhs", name="rhs")
    ones_bf = p_const.tile([P, P], bf16, tag="onesbf")
    ones = p_const.tile([P, 2, P], fp8, tag="ones")
    ident = p_const.tile([P, P], bf16, tag="ident")
    posT = p_stats.tile([P, NI], f32, tag="posT")
    posn = p_stats.tile([P, NI], f32, tag="posn")
    lnrow = p_stats.tile([1, ROWS_PER_CORE], f32, tag="lnrow")

    from concourse import library_config
    nc.gpsimd.load_library(library_config.proxy)
    nc.vector.memset(ones[:], 1.0)
    nc.vector.memset(ones_bf[:], 1.0)
    make_identity(nc, ident[:])

    # preload the Ln/Exp activation table set off the critical path: a dummy
    # Ln at t~0 forces the (single) table load before the prep chain needs it
    warm = p_const.tile([1, 1], f32, tag="warm")
    warm_o = p_const.tile([1, 1], f32, tag="warmo")
    nc.vector.memset(warm[:], 1.0)
    nc.vector.memset(ln2_c[:], float(np.log(2.0)))
    nc.vector.memset(neg_e2[:], -E2)
    nc.scalar.activation(warm_o[:], warm[:], Fn.Ln)
    nc.scalar.activation(warm[:], warm_o[:], Fn.Exp)

    cs = p_cs.tile([P, ROWS_PER_CORE], f32)

    # input loads in 8-tile pieces -- pure loads, no waits.  Issued from
    # three different engine queues (SP / ACT / DVE) so the ~1.2us
    # per-DMA sequencer issue time is paid in parallel, and emitted in an
    # order that leaves the shared DMA engines available for the small
    # dependent transfers early on.
    def load_piece(eng, k, which):
        t0, t1 = k * 8, (k + 1) * 8
        if which == 'zn':
            eng.dma_start(out=znat[:, t0:t1, :], in_=zn_ap[:, t0 * D:t1 * D]
                          .rearrange("p (t c) -> p t c", c=D))
        else:
            eng.dma_start(out=zT[:, :, t0 * P:t1 * P],
                          in_=zt_ap[:, :, t0 * P:t1 * P].rearrange(
                              "h p j -> p h j"))

    nc.sync.dma_start(out=znat[:, 0:4, :], in_=zn_ap[:, 0:4 * D]
                      .rearrange("p (t c) -> p t c", c=D))
    nc.sync.dma_start(out=znat[:, 4:8, :], in_=zn_ap[:, 4 * D:8 * D]
                      .rearrange("p (t c) -> p t c", c=D))
    load_piece(nc.sync, 0, 'zt')
    load_piece(nc.sync, 1, 'zn')
    load_piece(nc.sync, 1, 'zt')

    def prep_chunk(t0, t1, dve=False):
        # sum-of-squares pipeline; Pool (SBUF-only engine) for most chunks,
        # DVE (2x bf16, idle early) for the first ones
        if dve:
            nc.vector.tensor_tensor(sq[:, t0:t1, :], znat[:, t0:t1, :],
                                    znat[:, t0:t1, :], op=Op.mult)
        else:
            nc.gpsimd.tensor_tensor(sq[:, t0:t1, :], znat[:, t0:t1, :],
                                    znat[:, t0:t1, :], op=Op.mult)
        src = sq[:, t0:t1, :].rearrange("p t (two c) -> p t two c", two=2)
        levels = [(tl[0], src)]
        e = nc.vector if dve else nc.gpsimd
        e.tensor_tensor(tl[0][:, t0:t1, :], src[:, :, 0, :], src[:, :, 1, :],
                        op=Op.add)
        if dve:
            for k in range(4):
                s2 = tl[k][:, t0:t1, :].rearrange(
                    "p t (two c) -> p t two c", two=2)
                nc.vector.tensor_tensor(tl[k + 1][:, t0:t1, :], s2[:, :, 0, :],
                                        s2[:, :, 1, :], op=Op.add)
            nc.vector.tensor_reduce(ss[:, t0:t1], tl[4][:, t0:t1, :],
                                    axis=mybir.AxisListType.X, op=Op.add)
        else:
            for k in range(6):
                s2 = tl[k][:, t0:t1, :].rearrange(
                    "p t (two c) -> p t two c", two=2)
                nc.gpsimd.tensor_tensor(tl[k + 1][:, t0:t1, :], s2[:, :, 0, :],
                                        s2[:, :, 1, :], op=Op.add)
            s2 = tl[6][:, t0:t1, :]
            nc.gpsimd.tensor_tensor(
                ss[:, t0:t1].rearrange("p (t o) -> p t o", o=1),
                s2[:, :, 0:1], s2[:, :, 1:2], op=Op.add)
        # ACT: inv = exp(-0.5 ln ss)
        nc.scalar.activation(lns[:, t0:t1], ss[:, t0:t1], Fn.Ln)
        nc.scalar.activation(inv[:, t0:t1], lns[:, t0:t1], Fn.Exp, scale=-0.5)
        # DVE: per-tile Schraudolph scale
        nc.vector.tensor_scalar(a_col[:, t0:t1], inv[:, t0:t1], A_EXP, None,
                                op0=Op.mult)

    # first chunk covers the own rows -> enables rhs + the matmul stream.
    # NOTE: own columns are used in "pi order" col = p*8 + t (p = j % 128,
    # t = j // 128) so that inv_own can be row-ified by a plain DMA; the
    # column order of the S block / colsums is irrelevant to the final sum,
    # and the positives diagonal is recovered from a strided view.
    prep_chunk(*OWN_PIECES[0], dve=True)
    prep_chunk(*OWN_PIECES[1], dve=True)
    # 2/||z|| for the own rows: exp(-0.5 ln ss + ln 2); the rhs carries the
    # factor 2 of exp(2 S / tau_scale), so the per-tile ACT scale is plain inv
    nc.scalar.activation(inv2own[:], lns[:, 0:NI], Fn.Exp, scale=-0.5,
                         bias=ln2_c[:])
    # replicate inv2own across partitions WITHOUT a DMA hop: mask it with the
    # identity (pure broadcast views) and column-sum via a bf16 ones-matmul
    # into the cs PSUM banks (free until the first colsum accumulation, which
    # Tile orders after the rhs read below)
    nc.gpsimd.tensor_tensor(
        masked[:].rearrange("p (q t) -> p q t", t=NI),
        inv2own[:].rearrange("p (o t) -> p o t", o=1).broadcast_to(
            (P, P, NI)),
        ident[:].rearrange("p (q o) -> p q o", o=1).broadcast_to((P, P, NI)),
        op=Op.mult)
    for c in range(2):
        nc.tensor.matmul(cs[:, c * 512:(c + 1) * 512], lhsT=ones_bf[:],
                         rhs=masked[:, c * 512:(c + 1) * 512],
                         start=True, stop=True)
    nc.vector.tensor_tensor(
        rhs[:].rearrange("q h (p t) -> q h p t", t=NI),
        zT[:, :, 0:ROWS_PER_CORE].rearrange("q h (t p) -> q h p t", p=P),
        cs[:].rearrange("q (o p t) -> q o p t", o=1, t=NI).broadcast_to(
            (P, 2, P, NI)),
        op=Op.mult)

    ex_state = {}
    pend_cs = []

    def do_tile(t):
        s_ps = p_s.tile([P, ROWS_PER_CORE], f32, tag="s", name="s_ps")
        for c in range(2):
            nc.tensor.matmul(
                s_ps[:, c * 512:(c + 1) * 512],
                lhsT=zT[:, :, t * P:(t + 1) * P],
                rhs=rhs[:, :, c * 512:(c + 1) * 512],
                start=True, stop=True, perf_mode=DR)
        u, slot = divmod(t, 2)
        if slot == 0:
            ex = p_ex.tile([P, 2, ROWS_PER_CORE], fp8, tag="ex", name="ex")
            ex_state['ex'] = ex
        else:
            ex = ex_state['ex']
        eng = ASSIGN[t]
        if eng == 'A':
            nc.scalar.activation(ex[:, slot, :], s_ps[:], Fn.Exp,
                                 scale=inv[:, t:t + 1])
        else:
            e = nc.vector if eng == 'D' else nc.gpsimd
            e.tensor_scalar(ex[:, slot, :].bitcast(i8), s_ps[:],
                            a_col[:, t:t + 1], B_EXP, op0=Op.mult, op1=Op.add)
        if 32 <= t < 40:
            dump = p_dump.tile([P, P], f32, tag="dump", name="dump")
            k = t - 32
            # positives sit at (p, col p*8+k) in pi order: diagonal of the
            # strided view s_ps[p, m*8+k], extracted by identity-mask
            # multiply + row reduce
            nc.vector.tensor_tensor(
                dump[:],
                s_ps[:].rearrange("p (m t) -> p t m", t=NI)[:, k, :],
                ident[:], op=Op.mult)
            nc.vector.tensor_reduce(posT[:, k:k + 1], dump[:],
                                    axis=mybir.AxisListType.X, op=Op.add)
        if slot == 1:
            pend_cs.append((u, ex))
        # defer the colsum matmuls a few tiles so a lagging exp pair can't
        # stall the S matmuls behind it in PE's in-order queue
        while pend_cs and (pend_cs[0][0] * 2 + 9 <= t or t == NT - 1):
            uu, exx = pend_cs.pop(0)
            for c in range(2):
                nc.tensor.matmul(
                    cs[:, c * 512:(c + 1) * 512],
                    lhsT=ones[:], rhs=exx[:, :, c * 512:(c + 1) * 512],
                    start=(uu == 0), stop=(uu == NT // 2 - 1), perf_mode=DR)

    # remaining input pieces, spread across the SP / ACT / DVE queues
    # (issued after the chunk-0-critical work of each queue)
    # zn pieces first: they feed the serial sum-of-squares prep chain and
    # gate chunk readiness ~6us ahead of use, while zt pieces are only
    # needed at matmul time (far more slack)
    for eng, k, which in [(nc.sync, 2, 'zn'), (nc.sync, 3, 'zn'),
                          (nc.sync, 2, 'zt'), (nc.sync, 4, 'zn'),
                          (nc.sync, 3, 'zt'), (nc.sync, 5, 'zn'),
                          (nc.sync, 4, 'zt'), (nc.sync, 6, 'zn'),
                          (nc.sync, 7, 'zn'), (nc.sync, 5, 'zt'),
                          (nc.sync, 6, 'zt'), (nc.sync, 7, 'zt')]:
        load_piece(eng, k, which)

    # interleave prep of chunk c+1 into the tile stream of chunk c so each
    # engine's in-order queue alternates prep and exp work (prep emitted a
    # couple of tiles in, so the first tiles of a chunk aren't stuck behind
    # the next chunk's prep in the queues)
    prep_chunk(8, 15, dve=True)
    for ci, (t0, t1) in enumerate(CHUNKS):
        for t in range(t0, t1):
            do_tile(t)
            if ci == 0 and t == t0:
                prep_chunk(15, 22)
            if t == t0 + 1 and 1 <= ci + 1 < len(CHUNKS) and ci + 1 != 1:
                prep_chunk(*CHUNKS[ci + 1])

    # tail: lnrow = ln(colsum - e^2) in one ACT op (bias AP); positives
    # (posT = 2 inv_i G, so posn = 2 inv_i inv_j G and the host weights it
    # by -1 instead of -2)
    nc.scalar.activation(lnrow[:], cs[0:1, :], Fn.Ln, bias=neg_e2[:])
    nc.vector.tensor_tensor(posn[:], posT[:], inv[:, 32:40], op=Op.mult)
    nc.scalar.dma_start(out=lnrow_ap, in_=lnrow[:])
    nc.sync.dma_start(out=posn_ap, in_=posn[:])


def build_nc():
    """Build (once) the Bass module shared by all 8 cores."""
    from contextlib import ExitStack

    from concourse import bacc, mybir
    import concourse.tile as tile

    nc = bacc.Bacc("TRN2", target_bir_lowering=False, debug=False)
    fp8 = mybir.dt.float8e4
    zn = nc.dram_tensor("zn", [P, NT * D], mybir.dt.bfloat16,
                        kind="ExternalInput").ap()
    zt = nc.dram_tensor("zt", [2, P, N], fp8, kind="ExternalInput").ap()
    lnrow = nc.dram_tensor("lnrow", [1, ROWS_PER_CORE], mybir.dt.float32,
                           kind="ExternalOutput").ap()
    posn = nc.dram_tensor("posn", [P, NI], mybir.dt.float32,
                          kind="ExternalOutput").ap()
    with tile.TileContext(nc) as tc:
        with ExitStack() as ctx:
            _kernel_body(ctx, tc, lnrow, posn, zn, zt)
    return nc


_NC = None


def _get_nc(finalized=True):
    global _NC
    if _NC is None:
        _NC = build_nc()
    if finalized and not _NC.is_finalized():
        _NC.finalize()
    return _NC


def make_in_maps(z_orig, z_augment):
    from concourse import mybir

    f8np = mybir.dt.np(mybir.dt.float8e4)
    z = np.ascontiguousarray(
        np.concatenate([np.asarray(z_augment, dtype=np.float32),
                        np.asarray(z_orig, dtype=np.float32)], axis=0))
    maps = []
    for c in range(N_CORES):
        zr = np.roll(z, -ROWS_PER_CORE * c, axis=0)
        zf8 = zr.astype(f8np)
        zbf = zr.astype(mybir.dt.np(mybir.dt.bfloat16))
        # natural, pre-swizzled to SBUF layout: zn[p, t*256 + c] = z[t*128+p, c]
        znat = np.ascontiguousarray(
            zbf.reshape(NT, P, D).transpose(1, 0, 2).reshape(P, NT * D))
        # transposed: zt[h, p, j] = z[j, 128h + p]
        zt = np.ascontiguousarray(zf8.T.reshape(2, P, N))
        maps.append({"zn": znat, "zt": zt})
    return maps


def reduce_outputs(results):
    total = 0.0
    for r in results:
        total += float(np.asarray(r["lnrow"], dtype=np.float64).sum())
        total -= float(np.asarray(r["posn"], dtype=np.float64).sum())
    return np.float32(total / N)


def kernel(z_orig, z_augment):
    from concourse.bass_utils import run_bass_kernel_spmd

    nc = _get_nc()
    in_maps = make_in_maps(z_orig, z_augment)
    res = run_bass_kernel_spmd(nc, in_maps, core_ids=list(range(N_CORES)))
    return reduce_outputs(res.results)



# revision 10
# speedup vs baseline: 1.3316x; 1.3316x over previous
"""Trainium2 Bass kernel for nn_ContrastiveLoss2 (SimCLR NT-Xent loss).

Math (matches the jax reference):
    z  = concat([z_augment, z_orig])                       # [N=8192, D=256]
    zn = z / max(||z||, eps)                               # row L2 normalize
    S  = zn @ zn.T                                         # cosine sim [N, N]
    loss_i = -2 S[i, i+-B] + log( sum_{j != i} exp(2 S[i,j]) )
    out = mean_i loss_i                                    # tau = 0.5

Identity: denominator_i = sum_j exp(2 S_ij) - e^2 (S_ii == 1).

SYMMETRIC distribution (v2): S is symmetric, so each unordered pair is
computed once fleet-wide.  Core c (rows rolled so its own 1024 rows sit
at [0:1024)) computes the S block [5120 j-rows x 1024 own columns]:
  - j-tiles 0..31  (cores c..c+3):   full weight
  - j-tiles 32..39 (core c+4):       half weight, exp(2S - ln2); the
    partner core c+4 computes the transposed block also at half weight,
    so every pair still sums to 1.
Per-core outputs (denominators are assembled on the HOST):
  - csrow [2, 512]: column sums of exp over the block's 5120 j-rows
    (PE fp8 ones-matmul into PSUM) -> partial denominators for the
    core's own 1024 rows.  Row 0 holds cols 0:512, row 1 cols 512:1024.
  - rs [128, 32]: free-dim row sums of the exp tiles 8..39 -> partial
    denominators for rows owned by cores c+1..c+4.  ACT tiles get these
    free via the activation accumulator; Schraudolph tiles use a
    tensor_scalar+accum on DVE/Pool over the fp8 exp tile.
  - pose [128, 8]: the positive entries, read from the exp tiles
    (tiles 32..39 diagonal) by a Pool masked multiply+accum; the host
    inverts exp (or the Schraudolph bit pattern) to recover 2 S_pos.
Host: r_i = own colsum + 4 partner rowsums; loss = (sum ln(r_i - e^2)
- sum 2 S_pos) / N.

Engines: exp tiles split ACT (activation Exp, per-partition scale
inv_j) / DVE (Schraudolph: int8 = S*a_j + b bit-pattern IS fp8 exp);
norm prep: own rows via DVE tensor_tensor_reduce, j-rows via Pool
square + add-tree; PE: S matmuls + fp8 DoubleRow colsum accumulation.
"""

import sys

import numpy as np

try:
    import concourse  # noqa: F401
except ImportError:  # pragma: no cover
    sys.path.insert(0, "/opt/trn_rl_repo")

N_CORES = 8
N = 8192          # total rows (2B)
D = 256           # feature dim
B = 4096          # batch (positive offset)
P = 128           # SBUF partitions
NT = 40           # j-tiles per core (5/8 of 64)
NJ = NT * P       # 5120 j rows per core
RPC = 1024        # own columns per core
NI = RPC // P     # 8 own col-tiles
TAU = 0.5
E2 = float(np.exp(2.0))
LN2 = float(np.log(2.0))
A_EXP = 8.0 / LN2      # rhs carries the factor 2 -> a = inv * 8/ln2
SIGMA = 0.0435
# real-HW fp32->int8 convert rounds to nearest (the simulator truncates);
# calibrate for hardware, the graded correctness path
B_EXP = 56.0 - 8.0 * SIGMA

# per-tile exp engine assignment: A(CT) / D(VE Schraudolph)
ASSIGN = ['A', 'A', 'A', 'D', 'A', 'A', 'D', 'A'] + ['A', 'D'] * 16
# rowsum engine for DVE tiles (t >= 8): 'd' (DVE) early while Pool
# preps, 'p' (Pool) late
RSENG = {}
for _t in range(8, NT):
    if ASSIGN[_t] == 'D':
        RSENG[_t] = 'd' if _t < 24 else 'p'

# Pool prep chunks over j-tiles 8..39 (own tiles 0..7 use DVE ttr)
CHUNKS = [(8, 16), (16, 24), (24, 32), (32, 40)]


def _kernel_body(ctx, tc, csrow_ap, rs_ap, pose_ap, zn_ap, zt_ap):
    from concourse import mybir
    from concourse.masks import make_identity

    nc = tc.nc
    f32 = mybir.dt.float32
    bf16 = mybir.dt.bfloat16
    fp8 = mybir.dt.float8e4
    i8 = mybir.dt.int8
    Fn = mybir.ActivationFunctionType
    Op = mybir.AluOpType
    DR = mybir.MatmulPerfMode.DoubleRow

    p_const = ctx.enter_context(tc.tile_pool(name="const", bufs=1))
    p_z = ctx.enter_context(tc.tile_pool(name="z", bufs=1))
    p_sq = ctx.enter_context(tc.tile_pool(name="sq", bufs=1))
    p_tree = ctx.enter_context(tc.tile_pool(name="tree", bufs=1))
    p_stats = ctx.enter_context(tc.tile_pool(name="stats", bufs=1))
    p_ex = ctx.enter_context(tc.tile_pool(name="ex", bufs=6))
    p_junk = ctx.enter_context(tc.tile_pool(name="junk", bufs=2))
    p_s = ctx.enter_context(tc.tile_pool(name="s", bufs=3, space="PSUM"))
    p_cs = ctx.enter_context(tc.tile_pool(name="cs", bufs=1, space="PSUM"))

    znat = p_z.tile([P, NT, D], bf16, tag="znat", name="znat")
    zT = p_z.tile([P, 2, NJ], fp8, tag="zT", name="zT")
    sq = p_sq.tile([P, 32, D], bf16, tag="sq")
    # tree levels for Pool chunks: widths 128 ... 2 bf16, final add -> f32
    tl = [p_tree.tile([P, 32, D // (2 << k)], bf16, tag=f"tl{k}", name=f"tl{k}")
          for k in range(7)]
    sqj = p_sq.tile([P, D], bf16, tag="sqj")  # ttr junk out (own prep)
    ss = p_stats.tile([P, NT], f32, tag="ss")
    lns = p_stats.tile([P, NT], f32, tag="lns")
    inv = p_stats.tile([P, NT], f32, tag="inv")
    a_col = p_stats.tile([P, NT], f32, tag="a_col")
    inv2own = p_stats.tile([P, NI], f32, tag="inv2own")
    ln2_c = p_const.tile([P, 1], f32, tag="ln2c")
    nln2_c = p_const.tile([P, 1], f32, tag="nln2c")
    masked = p_stats.tile([P, RPC], bf16, tag="masked")
    rhs = p_z.tile([P, 2, RPC], fp8, tag="rhs", name="rhs")
    ones_bf = p_const.tile([P, P], bf16, tag="onesbf")
    ones = p_const.tile([P, 2, P], fp8, tag="ones")
    ident = p_const.tile([P, P], bf16, tag="ident")
    rs = p_stats.tile([P, 32], f32, tag="rs")
    posE = p_stats.tile([P, NI], f32, tag="posE")
    csrow = p_stats.tile([1, RPC], f32, tag="csrow")

    from concourse import library_config
    nc.gpsimd.load_library(library_config.proxy)

    # input DMAs, spread across queues; zT0 on ACT, zT1 on DVE so the SP
    # queue's serial issue stream starts with the prep-critical zn pieces
    nc.scalar.dma_start(out=zT[:, :, 0:1024],
                        in_=zt_ap[:, :, 0:1024].rearrange("h p j -> p h j"))
    nc.scalar.dma_start(out=zT[:, :, 1024:2048],
                        in_=zt_ap[:, :, 1024:2048].rearrange("h p j -> p h j"))

    def load_zn(eng, t0, t1):
        eng.dma_start(out=znat[:, t0:t1, :], in_=zn_ap[:, t0 * D:t1 * D]
                      .rearrange("p (t c) -> p t c", c=D))

    def load_zt(eng, k):
        eng.dma_start(out=zT[:, :, k * 1024:(k + 1) * 1024],
                      in_=zt_ap[:, :, k * 1024:(k + 1) * 1024]
                      .rearrange("h p j -> p h j"))

    load_zn(nc.sync, 0, 4)
    load_zn(nc.sync, 4, 8)
    load_zn(nc.sync, 8, 16)
    load_zt(nc.sync, 2)
    load_zn(nc.sync, 16, 24)
    load_zt(nc.sync, 3)
    load_zn(nc.sync, 24, 32)
    load_zt(nc.sync, 4)
    load_zn(nc.sync, 32, 40)

    # consts (DVE memsets are cheap; ident is built on Pool)
    nc.vector.memset(ones[:], 1.0)
    nc.vector.memset(ones_bf[:], 1.0)
    nc.vector.memset(ln2_c[:], LN2)
    nc.vector.memset(nln2_c[:], -LN2)
    make_identity(nc, ident[:])

    # preload the Ln/Exp activation table off the critical path
    warm = p_const.tile([1, 1], f32, tag="warm")
    warm_o = p_const.tile([1, 1], f32, tag="warmo")
    nc.gpsimd.memset(warm[:], 1.0)
    nc.scalar.activation(warm_o[:], warm[:], Fn.Ln)

    cs = p_cs.tile([P, RPC], f32)

    # --- own-row prep: ttr (fused square + row reduce) on DVE, 2 halves ---
    def own_prep(h0, h1):
        for t in range(h0, h1):
            nc.vector.tensor_tensor_reduce(
                sqj[:], znat[:, t, :], znat[:, t, :], 1.0, 0.0,
                op0=Op.mult, op1=Op.add, accum_out=ss[:, t:t + 1])
        nc.scalar.activation(lns[:, h0:h1], ss[:, h0:h1], Fn.Ln)
        nc.scalar.activation(inv[:, h0:h1], lns[:, h0:h1], Fn.Exp, scale=-0.5)
        # 2/||z|| for the own rows (rhs carries the factor 2)
        nc.scalar.activation(inv2own[:, h0:h1], lns[:, h0:h1], Fn.Exp,
                             scale=-0.5, bias=ln2_c[:])
        # replicate inv2own across partitions: identity-mask (broadcast
        # views) + bf16 ones-matmul column sums into the cs PSUM banks
        # (free until the first colsum accumulation)
        nc.gpsimd.scalar_tensor_tensor(
            masked[:, h0 * P:h1 * P].rearrange("p (t q) -> p t q", q=P),
            inv2own[:, h0:h1].rearrange("p (t o) -> p t o", o=1)
            .broadcast_to((P, h1 - h0, P)),
            1.0,
            ident[:].rearrange("p (o q) -> p o q", o=1)
            .broadcast_to((P, h1 - h0, P)),
            op0=Op.mult, op1=Op.mult)
        nc.tensor.matmul(cs[:, h0 * P:h1 * P], lhsT=ones_bf[:],
                         rhs=masked[:, h0 * P:h1 * P], start=True, stop=True)
        nc.vector.tensor_tensor(
            rhs[:, :, h0 * P:h1 * P],
            zT[:, :, h0 * P:h1 * P],
            cs[:, h0 * P:h1 * P].rearrange("p (o c) -> p o c", o=1)
            .broadcast_to((P, 2, (h1 - h0) * P)),
            op=Op.mult)

    own_prep(0, 4)
    own_prep(4, 8)

    # --- Pool prep for j-tile chunks 8..39: square + binary add-tree ---
    def chunk_sq(t0, t1):
        nc.gpsimd.tensor_tensor(sq[:, t0 - 8:t1 - 8, :], znat[:, t0:t1, :],
                                znat[:, t0:t1, :], op=Op.mult)
        src = sq[:, t0 - 8:t1 - 8, :].rearrange(
            "p t (two c) -> p t two c", two=2)
        nc.gpsimd.tensor_tensor(tl[0][:, t0 - 8:t1 - 8, :], src[:, :, 0, :],
                                src[:, :, 1, :], op=Op.add)

    def chunk_tree(t0, t1, k0, k1):
        for k in range(k0, k1):
            s2 = tl[k][:, t0 - 8:t1 - 8, :].rearrange(
                "p t (two c) -> p t two c", two=2)
            nc.gpsimd.tensor_tensor(tl[k + 1][:, t0 - 8:t1 - 8, :],
                                    s2[:, :, 0, :], s2[:, :, 1, :], op=Op.add)

    def chunk_fin(t0, t1):
        s2 = tl[6][:, t0 - 8:t1 - 8, :]
        nc.gpsimd.tensor_tensor(
            ss[:, t0:t1].rearrange("p (t o) -> p t o", o=1),
            s2[:, :, 0:1], s2[:, :, 1:2], op=Op.add)
        nc.scalar.activation(lns[:, t0:t1], ss[:, t0:t1], Fn.Ln)
        nc.scalar.activation(inv[:, t0:t1], lns[:, t0:t1], Fn.Exp, scale=-0.5)
        nc.vector.tensor_scalar(a_col[:, t0:t1], inv[:, t0:t1], A_EXP, None,
                                op0=Op.mult)

    # a_col for own tiles (DVE Schraudolph scale)
    nc.vector.tensor_scalar(a_col[:, 0:8], inv[:, 0:8], A_EXP, None,
                            op0=Op.mult)

    ex_state = {}
    pend_cs = []

    def do_tile(t):
        s_ps = p_s.tile([P, RPC], f32, tag="s", name="s_ps")
        for c in range(2):
            nc.tensor.matmul(
                s_ps[:, c * 512:(c + 1) * 512],
                lhsT=zT[:, :, t * P:(t + 1) * P],
                rhs=rhs[:, :, c * 512:(c + 1) * 512],
                start=True, stop=True, perf_mode=DR)
        u, slot = divmod(t, 2)
        if slot == 0:
            ex = p_ex.tile([P, 2, RPC], fp8, tag="ex", name="ex")
            ex_state['ex'] = ex
        else:
            ex = ex_state['ex']
        half = t >= 32
        if ASSIGN[t] == 'A':
            kw = {}
            if t >= 8:
                kw['accum_out'] = rs[:, t - 8:t - 7]
            if half:
                kw['bias'] = nln2_c[:]
            nc.scalar.activation(ex[:, slot, :], s_ps[:], Fn.Exp,
                                 scale=inv[:, t:t + 1], **kw)
        else:
            b = B_EXP - (8.0 if half else 0.0)
            nc.vector.tensor_scalar(ex[:, slot, :].bitcast(i8), s_ps[:],
                                    a_col[:, t:t + 1], b,
                                    op0=Op.mult, op1=Op.add)
            if t >= 8:
                eng = nc.vector if RSENG[t] == 'd' else nc.gpsimd
                junk = p_junk.tile([P, RPC], bf16, tag="junk", name="junk")
                eng.tensor_scalar(junk[:], ex[:, slot, :], 1.0, None,
                                  op0=Op.mult, op1=Op.add,
                                  accum_out=rs[:, t - 8:t - 7])
        if half:
            # positive entries: diagonal of col-block (t-32) of the exp
            # tile, via Pool masked multiply + accumulate
            k = t - 32
            junkm = p_junk.tile([P, P], bf16, tag="junkm", name="junkm")
            nc.gpsimd.scalar_tensor_tensor(
                junkm[:], ex[:, slot, k * P:(k + 1) * P], 1.0, ident[:],
                op0=Op.mult, op1=Op.mult, accum_out=posE[:, k:k + 1])
        if slot == 1:
            pend_cs.append((u, ex))
        # defer colsum matmuls so a lagging exp pair can't stall the S
        # matmuls behind it in PE's in-order queue
        while pend_cs and (pend_cs[0][0] * 2 + 7 <= t or t == NT - 1):
            uu, exx = pend_cs.pop(0)
            for c in range(2):
                nc.tensor.matmul(
                    cs[:, c * 512:(c + 1) * 512],
                    lhsT=ones[:], rhs=exx[:, :, c * 512:(c + 1) * 512],
                    start=(uu == 0), stop=(uu == NT // 2 - 1), perf_mode=DR)

    # interleave Pool chunk prep into the tile stream so chunk c's ops
    # don't block the Pool rowsum/pos ops of earlier tiles
    PREP_AT = {
        0: lambda: chunk_sq(8, 16),
        1: lambda: chunk_tree(8, 16, 0, 3),
        2: lambda: chunk_tree(8, 16, 3, 6),
        3: lambda: chunk_fin(8, 16),
        4: lambda: chunk_sq(16, 24),
        6: lambda: chunk_tree(16, 24, 0, 3),
        8: lambda: chunk_tree(16, 24, 3, 6),
        10: lambda: chunk_fin(16, 24),
        12: lambda: chunk_sq(24, 32),
        14: lambda: chunk_tree(24, 32, 0, 3),
        16: lambda: chunk_tree(24, 32, 3, 6),
        18: lambda: chunk_fin(24, 32),
        20: lambda: chunk_sq(32, 40),
        22: lambda: chunk_tree(32, 40, 0, 3),
        24: lambda: chunk_tree(32, 40, 3, 6),
        26: lambda: chunk_fin(32, 40),
    }
    for t in range(NT):
        do_tile(t)
        if t in PREP_AT:
            PREP_AT[t]()

    # tail: cs row readout split ACT/DVE (the 128 cs rows are identical)
    nc.vector.tensor_scalar(csrow[:, 0:512], cs[0:1, 0:512], 1.0, None,
                            op0=Op.mult)
    nc.scalar.activation(csrow[:, 512:1024], cs[0:1, 512:1024], Fn.Copy)
    nc.sync.dma_start(out=rs_ap, in_=rs[:])
    nc.scalar.dma_start(out=pose_ap, in_=posE[:])
    nc.sync.dma_start(out=csrow_ap, in_=csrow[:])


def build_nc():
    """Build (once) the Bass module shared by all 8 cores."""
    from contextlib import ExitStack

    from concourse import bacc, mybir
    import concourse.tile as tile

    nc = bacc.Bacc("TRN2", target_bir_lowering=False, debug=False)
    fp8 = mybir.dt.float8e4
    zn = nc.dram_tensor("zn", [P, NT * D], mybir.dt.bfloat16,
                        kind="ExternalInput").ap()
    zt = nc.dram_tensor("zt", [2, P, NJ], fp8, kind="ExternalInput").ap()
    csrow = nc.dram_tensor("csrow", [1, RPC], mybir.dt.float32,
                           kind="ExternalOutput").ap()
    rs = nc.dram_tensor("rs", [P, 32], mybir.dt.float32,
                        kind="ExternalOutput").ap()
    pose = nc.dram_tensor("pose", [P, NI], mybir.dt.float32,
                          kind="ExternalOutput").ap()
    with tile.TileContext(nc) as tc:
        with ExitStack() as ctx:
            _kernel_body(ctx, tc, csrow, rs, pose, zn, zt)
    return nc


_NC = None


def _get_nc(finalized=True):
    global _NC
    if _NC is None:
        _NC = build_nc()
    if finalized and not _NC.is_finalized():
        _NC.finalize()
    return _NC


def make_in_maps(z_orig, z_augment):
    from concourse import mybir

    f8np = mybir.dt.np(mybir.dt.float8e4)
    bfnp = mybir.dt.np(mybir.dt.bfloat16)
    z = np.ascontiguousarray(
        np.concatenate([np.asarray(z_augment, dtype=np.float32),
                        np.asarray(z_orig, dtype=np.float32)], axis=0))
    maps = []
    for c in range(N_CORES):
        zr = np.roll(z, -RPC * c, axis=0)[:NJ]
        zf8 = zr.astype(f8np)
        zbf = zr.astype(bfnp)
        # natural, pre-swizzled: zn[p, t*256 + c] = z[t*128+p, c]
        znat = np.ascontiguousarray(
            zbf.reshape(NT, P, D).transpose(1, 0, 2).reshape(P, NT * D))
        # transposed: zt[h, p, j] = z[j, 128h + p]
        zt = np.ascontiguousarray(zf8.T.reshape(2, P, NJ))
        maps.append({"zn": znat, "zt": zt})
    return maps


def reduce_outputs(results):
    """Host assembly: denominators from colsum + rowsums, ln, positives."""
    r = np.zeros(N, dtype=np.float64)
    pos_total = 0.0
    for c, res in enumerate(results):
        base = RPC * c
        cs = np.asarray(res["csrow"], dtype=np.float64).reshape(RPC)
        idx = (base + np.arange(RPC)) % N
        r[idx] += cs
        rsv = np.asarray(res["rs"], dtype=np.float64)  # [128, 32]
        for t in range(8, NT):
            jdx = (base + P * t + np.arange(P)) % N
            r[jdx] += rsv[:, t - 8]
        # positives: decode the exp-tile values back to 2 S_cos
        pe = np.asarray(res["pose"], dtype=np.float32)  # [128, 8]
        for k in range(NI):
            t = 32 + k
            v = pe[:, k].astype(np.float64)
            if ASSIGN[t] == 'A':
                # v = fp8(exp(2S - ln2)) -> 2S = ln(2 v)
                twos = np.log(np.maximum(2.0 * v, 1e-30))
            else:
                # v = fp8-bit-pattern Schraudolph: bits = 2S*8/ln2 + B - 8
                from concourse import mybir
                f8np = mybir.dt.np(mybir.dt.float8e4)
                bits = pe[:, k].astype(f8np).view(np.uint8).astype(np.float64)
                twos = (bits - (B_EXP - 8.0)) * LN2 / 8.0
            pos_total += float(twos.sum())
    total = float(np.log(np.maximum(r - E2, 1e-300)).sum()) - pos_total
    return np.float32(total / N)


def kernel(z_orig, z_augment):
    from concourse.bass_utils import run_bass_kernel_spmd

    nc = _get_nc()
    in_maps = make_in_maps(z_orig, z_augment)
    res = run_bass_kernel_spmd(nc, in_maps, core_ids=list(range(N_CORES)))
    return reduce_outputs(res.results)


# revision 14
# speedup vs baseline: 1.3514x; 1.0149x over previous
"""Trainium2 Bass kernel for nn_ContrastiveLoss2 (SimCLR NT-Xent loss).

Math (matches the jax reference):
    z  = concat([z_augment, z_orig])                       # [N=8192, D=256]
    zn = z / max(||z||, eps)                               # row L2 normalize
    S  = zn @ zn.T                                         # cosine sim [N, N]
    loss_i = -2 S[i, i+-B] + log( sum_{j != i} exp(2 S[i,j]) )
    out = mean_i loss_i                                    # tau = 0.5

Identity: denominator_i = sum_j exp(2 S_ij) - e^2 (S_ii == 1).

SYMMETRIC distribution (v2): S is symmetric, so each unordered pair is
computed once fleet-wide.  Core c (rows rolled so its own 1024 rows sit
at [0:1024)) computes the S block [5120 j-rows x 1024 own columns]:
  - j-tiles 0..31  (cores c..c+3):   full weight
  - j-tiles 32..39 (core c+4):       half weight, exp(2S - ln2); the
    partner core c+4 computes the transposed block also at half weight,
    so every pair still sums to 1.
Per-core outputs (denominators are assembled on the HOST):
  - csrow [2, 512]: column sums of exp over the block's 5120 j-rows
    (PE fp8 ones-matmul into PSUM) -> partial denominators for the
    core's own 1024 rows.  Row 0 holds cols 0:512, row 1 cols 512:1024.
  - rs [128, 32]: free-dim row sums of the exp tiles 8..39 -> partial
    denominators for rows owned by cores c+1..c+4.  ACT tiles get these
    free via the activation accumulator; Schraudolph tiles use a
    tensor_scalar+accum on DVE/Pool over the fp8 exp tile.
  - pose [128, 8]: the positive entries, read from the exp tiles
    (tiles 32..39 diagonal) by a Pool masked multiply+accum; the host
    inverts exp (or the Schraudolph bit pattern) to recover 2 S_pos.
Host: r_i = own colsum + 4 partner rowsums; loss = (sum ln(r_i - e^2)
- sum 2 S_pos) / N.

Engines: exp tiles split ACT (activation Exp, per-partition scale
inv_j) / DVE (Schraudolph: int8 = S*a_j + b bit-pattern IS fp8 exp);
norm prep: own rows via DVE tensor_tensor_reduce, j-rows via Pool
square + add-tree; PE: S matmuls + fp8 DoubleRow colsum accumulation.
"""

import sys

import numpy as np

try:
    import concourse  # noqa: F401
except ImportError:  # pragma: no cover
    sys.path.insert(0, "/opt/trn_rl_repo")

N_CORES = 8
N = 8192          # total rows (2B)
D = 256           # feature dim
B = 4096          # batch (positive offset)
P = 128           # SBUF partitions
NT = 40           # j-tiles per core (5/8 of 64)
NJ = NT * P       # 5120 j rows per core
RPC = 1024        # own columns per core
NI = RPC // P     # 8 own col-tiles
TAU = 0.5
E2 = float(np.exp(2.0))
LN2 = float(np.log(2.0))
A_EXP = 8.0 / LN2      # rhs carries the factor 2 -> a = inv * 8/ln2
SIGMA = 0.0435
# real-HW fp32->int8 convert rounds to nearest (the simulator truncates);
# calibrate for hardware, the graded correctness path
B_EXP = 56.0 - 8.0 * SIGMA

# per-tile exp engine assignment: A(CT) / D(VE Schraudolph)
ASSIGN = ['A', 'A', 'A', 'D', 'A', 'A', 'D', 'A'] + ['A', 'D'] * 16

# Pool prep chunks over j-tiles 8..39 (own tiles 0..7 use DVE ttr)
CHUNKS = [(8, 16), (16, 24), (24, 32), (32, 40)]


def _kernel_body(ctx, tc, csrow_ap, rs_ap, pose_ap, zn_ap, zt_ap):
    from concourse import mybir
    from concourse.masks import make_identity

    nc = tc.nc
    f32 = mybir.dt.float32
    bf16 = mybir.dt.bfloat16
    fp8 = mybir.dt.float8e4
    i8 = mybir.dt.int8
    Fn = mybir.ActivationFunctionType
    Op = mybir.AluOpType
    DR = mybir.MatmulPerfMode.DoubleRow

    p_const = ctx.enter_context(tc.tile_pool(name="const", bufs=1))
    p_z = ctx.enter_context(tc.tile_pool(name="z", bufs=1))
    p_sq = ctx.enter_context(tc.tile_pool(name="sq", bufs=1))
    p_tree = ctx.enter_context(tc.tile_pool(name="tree", bufs=1))
    p_stats = ctx.enter_context(tc.tile_pool(name="stats", bufs=1))
    p_ex = ctx.enter_context(tc.tile_pool(name="ex", bufs=6))
    p_junk = ctx.enter_context(tc.tile_pool(name="junk", bufs=2))
    p_s = ctx.enter_context(tc.tile_pool(name="s", bufs=3, space="PSUM"))
    p_cs = ctx.enter_context(tc.tile_pool(name="cs", bufs=1, space="PSUM"))

    znat = p_z.tile([P, NT, D], bf16, tag="znat", name="znat")
    zT = p_z.tile([P, 2, NJ], fp8, tag="zT", name="zT")
    sq = p_sq.tile([P, 32, D], bf16, tag="sq")
    # tree levels for Pool chunks: widths 128 ... 2 bf16, final add -> f32
    tl = [p_tree.tile([P, 32, D // (2 << k)], bf16, tag=f"tl{k}", name=f"tl{k}")
          for k in range(7)]
    sqj = p_sq.tile([P, D], bf16, tag="sqj")  # ttr junk out (own prep)
    ss = p_stats.tile([P, NT], f32, tag="ss")
    lns = p_stats.tile([P, NT], f32, tag="lns")
    inv = p_stats.tile([P, NT], f32, tag="inv")
    a_col = p_stats.tile([P, NT], f32, tag="a_col")
    inv2own = p_stats.tile([P, NI], f32, tag="inv2own")
    ln2_c = p_const.tile([P, 1], f32, tag="ln2c")
    nln2_c = p_const.tile([P, 1], f32, tag="nln2c")
    masked = p_stats.tile([P, RPC], bf16, tag="masked")
    rhs = p_z.tile([P, 2, RPC], fp8, tag="rhs", name="rhs")
    ones_bf = p_const.tile([P, P], bf16, tag="onesbf")
    ones = p_const.tile([P, 2, P], fp8, tag="ones")
    ident = p_const.tile([P, P], bf16, tag="ident")
    rs = p_stats.tile([P, 32], f32, tag="rs")
    posE = p_stats.tile([P, NI], f32, tag="posE")
    csrow = p_stats.tile([1, RPC], f32, tag="csrow")

    from concourse import library_config
    nc.gpsimd.load_library(library_config.proxy)

    # input DMAs, spread across queues; zT0 on ACT, zT1 on DVE so the SP
    # queue's serial issue stream starts with the prep-critical zn pieces
    def load_zn(eng, t0, t1):
        eng.dma_start(out=znat[:, t0:t1, :], in_=zn_ap[:, t0 * D:t1 * D]
                      .rearrange("p (t c) -> p t c", c=D))

    def load_zt(eng, k):
        eng.dma_start(out=zT[:, :, k * 1024:(k + 1) * 1024],
                      in_=zt_ap[:, :, k * 1024:(k + 1) * 1024]
                      .rearrange("h p j -> p h j"))

    load_zt(nc.scalar, 0)
    load_zn(nc.sync, 0, 4)
    load_zn(nc.sync, 4, 8)
    load_zn(nc.sync, 8, 16)
    load_zt(nc.sync, 1)
    load_zn(nc.sync, 16, 24)
    load_zt(nc.sync, 2)
    load_zn(nc.sync, 24, 32)
    load_zt(nc.sync, 3)
    load_zn(nc.sync, 32, 40)
    load_zt(nc.sync, 4)

    # consts (DVE memsets are cheap; ident is built on Pool)
    nc.vector.memset(ones[:], 1.0)
    nc.vector.memset(ones_bf[:], 1.0)
    nc.vector.memset(ln2_c[:], LN2)
    nc.vector.memset(nln2_c[:], -LN2)
    make_identity(nc, ident[:])

    # preload the Ln/Exp activation table off the critical path
    warm = p_const.tile([1, 1], f32, tag="warm")
    warm_o = p_const.tile([1, 1], f32, tag="warmo")
    nc.gpsimd.memset(warm[:], 1.0)
    nc.scalar.activation(warm_o[:], warm[:], Fn.Ln)

    cs = p_cs.tile([P, RPC], f32)

    # --- own-row prep: ttr (fused square + row reduce) on DVE, 2 halves ---
    def own_prep(h0, h1):
        for t in range(h0, h1):
            nc.vector.tensor_tensor_reduce(
                sqj[:], znat[:, t, :], znat[:, t, :], 1.0, 0.0,
                op0=Op.mult, op1=Op.add, accum_out=ss[:, t:t + 1])
        nc.scalar.activation(lns[:, h0:h1], ss[:, h0:h1], Fn.Ln)
        nc.scalar.activation(inv[:, h0:h1], lns[:, h0:h1], Fn.Exp, scale=-0.5)
        # 2/||z|| for the own rows (rhs carries the factor 2)
        nc.scalar.activation(inv2own[:, h0:h1], lns[:, h0:h1], Fn.Exp,
                             scale=-0.5, bias=ln2_c[:])
        # replicate inv2own across partitions: identity-mask (broadcast
        # views) + bf16 ones-matmul column sums into the cs PSUM banks
        # (free until the first colsum accumulation)
        nc.gpsimd.tensor_tensor(
            masked[:, h0 * P:h1 * P].rearrange("p (t q) -> p t q", q=P),
            inv2own[:, h0:h1].rearrange("p (t o) -> p t o", o=1)
            .broadcast_to((P, h1 - h0, P)),
            ident[:].rearrange("p (o q) -> p o q", o=1)
            .broadcast_to((P, h1 - h0, P)),
            op=Op.mult)
        nc.tensor.matmul(cs[:, h0 * P:h1 * P], lhsT=ones_bf[:],
                         rhs=masked[:, h0 * P:h1 * P], start=True, stop=True)
        nc.vector.tensor_tensor(
            rhs[:, :, h0 * P:h1 * P],
            zT[:, :, h0 * P:h1 * P],
            cs[:, h0 * P:h1 * P].rearrange("p (o c) -> p o c", o=1)
            .broadcast_to((P, 2, (h1 - h0) * P)),
            op=Op.mult)

    own_prep(0, 4)
    own_prep(4, 8)

    # --- Pool prep for j-tile chunks 8..39: square + binary add-tree ---
    def chunk_sq(t0, t1):
        nc.gpsimd.tensor_tensor(sq[:, t0 - 8:t1 - 8, :], znat[:, t0:t1, :],
                                znat[:, t0:t1, :], op=Op.mult)
        src = sq[:, t0 - 8:t1 - 8, :].rearrange(
            "p t (two c) -> p t two c", two=2)
        nc.gpsimd.tensor_tensor(tl[0][:, t0 - 8:t1 - 8, :], src[:, :, 0, :],
                                src[:, :, 1, :], op=Op.add)

    def chunk_tree(t0, t1, k0, k1):
        for k in range(k0, k1):
            s2 = tl[k][:, t0 - 8:t1 - 8, :].rearrange(
                "p t (two c) -> p t two c", two=2)
            nc.gpsimd.tensor_tensor(tl[k + 1][:, t0 - 8:t1 - 8, :],
                                    s2[:, :, 0, :], s2[:, :, 1, :], op=Op.add)

    def chunk_fin(t0, t1):
        s2 = tl[6][:, t0 - 8:t1 - 8, :]
        nc.gpsimd.tensor_tensor(
            ss[:, t0:t1].rearrange("p (t o) -> p t o", o=1),
            s2[:, :, 0:1], s2[:, :, 1:2], op=Op.add)
        nc.scalar.activation(lns[:, t0:t1], ss[:, t0:t1], Fn.Ln)
        nc.scalar.activation(inv[:, t0:t1], lns[:, t0:t1], Fn.Exp, scale=-0.5)
        nc.vector.tensor_scalar(a_col[:, t0:t1], inv[:, t0:t1], A_EXP, None,
                                op0=Op.mult)

    # a_col for own tiles (DVE Schraudolph scale)
    nc.vector.tensor_scalar(a_col[:, 0:8], inv[:, 0:8], A_EXP, None,
                            op0=Op.mult)

    ex_state = {}
    pend_cs = []

    def do_tile(t):
        s_ps = p_s.tile([P, RPC], f32, tag="s", name="s_ps")
        for c in range(2):
            nc.tensor.matmul(
                s_ps[:, c * 512:(c + 1) * 512],
                lhsT=zT[:, :, t * P:(t + 1) * P],
                rhs=rhs[:, :, c * 512:(c + 1) * 512],
                start=True, stop=True, perf_mode=DR)
        u, slot = divmod(t, 2)
        if slot == 0:
            ex = p_ex.tile([P, 2, RPC], fp8, tag="ex", name="ex")
            ex_state['ex'] = ex
        else:
            ex = ex_state['ex']
        half = t >= 32
        if ASSIGN[t] == 'A':
            kw = {}
            if t >= 8:
                kw['accum_out'] = rs[:, t - 8:t - 7]
            if half:
                kw['bias'] = nln2_c[:]
            nc.scalar.activation(ex[:, slot, :], s_ps[:], Fn.Exp,
                                 scale=inv[:, t:t + 1], **kw)
        else:
            b = B_EXP - (8.0 if half else 0.0)
            nc.vector.tensor_scalar(ex[:, slot, :].bitcast(i8), s_ps[:],
                                    a_col[:, t:t + 1], b,
                                    op0=Op.mult, op1=Op.add)
            if t >= 8:
                # rowsum: Pool folds the fp8 tile 1024 -> 64 with 4 TT
                # adds (no Pool tensor_scalar on real HW); DVE finishes
                # with a 2x tensor_scalar + accumulate
                exs = ex[:, slot, :]
                fold = p_junk.tile([P, 960], bf16, tag="fold", name="fold")
                nc.gpsimd.tensor_tensor(fold[:, 0:512], exs[0:P, 0:512],
                                        exs[0:P, 512:1024], op=Op.add)
                nc.gpsimd.tensor_tensor(fold[:, 512:768], fold[:, 0:256],
                                        fold[:, 256:512], op=Op.add)
                nc.gpsimd.tensor_tensor(fold[:, 768:896], fold[:, 512:640],
                                        fold[:, 640:768], op=Op.add)
                nc.gpsimd.tensor_tensor(fold[:, 896:960], fold[:, 768:832],
                                        fold[:, 832:896], op=Op.add)
                junk = p_junk.tile([P, 64], bf16, tag="junk", name="junk")
                nc.vector.tensor_scalar(junk[:], fold[:, 896:960], 1.0, None,
                                        op0=Op.mult, op1=Op.add,
                                        accum_out=rs[:, t - 8:t - 7])
        if half:
            # positive entries: diagonal of col-block (t-32) of the exp
            # tile, via DVE masked multiply + accumulate (all-SBUF 2x)
            k = t - 32
            junkm = p_junk.tile([P, P], bf16, tag="junkm", name="junkm")
            nc.vector.scalar_tensor_tensor(
                junkm[:], ex[:, slot, k * P:(k + 1) * P], 1.0, ident[:],
                op0=Op.mult, op1=Op.mult, accum_out=posE[:, k:k + 1])
        if slot == 1:
            pend_cs.append((u, ex))
        # defer colsum matmuls so a lagging exp pair can't stall the S
        # matmuls behind it in PE's in-order queue
        while pend_cs and (pend_cs[0][0] * 2 + 7 <= t or t == NT - 1):
            uu, exx = pend_cs.pop(0)
            for c in range(2):
                nc.tensor.matmul(
                    cs[:, c * 512:(c + 1) * 512],
                    lhsT=ones[:], rhs=exx[:, :, c * 512:(c + 1) * 512],
                    start=(uu == 0), stop=(uu == NT // 2 - 1), perf_mode=DR)

    # interleave Pool chunk prep into the tile stream so chunk c's ops
    # don't block the Pool rowsum/pos ops of earlier tiles
    PREP_AT = {
        0: lambda: chunk_sq(8, 16),
        1: lambda: chunk_tree(8, 16, 0, 3),
        2: lambda: chunk_tree(8, 16, 3, 6),
        3: lambda: chunk_fin(8, 16),
        4: lambda: chunk_sq(16, 24),
        6: lambda: chunk_tree(16, 24, 0, 3),
        8: lambda: chunk_tree(16, 24, 3, 6),
        10: lambda: chunk_fin(16, 24),
        12: lambda: chunk_sq(24, 32),
        14: lambda: chunk_tree(24, 32, 0, 3),
        16: lambda: chunk_tree(24, 32, 3, 6),
        18: lambda: chunk_fin(24, 32),
        20: lambda: chunk_sq(32, 40),
        22: lambda: chunk_tree(32, 40, 0, 3),
        24: lambda: chunk_tree(32, 40, 3, 6),
        26: lambda: chunk_fin(32, 40),
    }
    for t in range(NT):
        do_tile(t)
        if t in PREP_AT:
            PREP_AT[t]()

    # tail: cs row readout split ACT/DVE (the 128 cs rows are identical)
    nc.vector.tensor_scalar(csrow[:, 0:512], cs[0:1, 0:512], 1.0, None,
                            op0=Op.mult)
    nc.scalar.activation(csrow[:, 512:1024], cs[0:1, 512:1024], Fn.Copy)
    nc.sync.dma_start(out=rs_ap, in_=rs[:])
    nc.scalar.dma_start(out=pose_ap, in_=posE[:])
    nc.sync.dma_start(out=csrow_ap, in_=csrow[:])


def build_nc():
    """Build (once) the Bass module shared by all 8 cores."""
    from contextlib import ExitStack

    from concourse import bacc, mybir
    import concourse.tile as tile

    nc = bacc.Bacc("TRN2", target_bir_lowering=False, debug=False)
    fp8 = mybir.dt.float8e4
    zn = nc.dram_tensor("zn", [P, NT * D], mybir.dt.bfloat16,
                        kind="ExternalInput").ap()
    zt = nc.dram_tensor("zt", [2, P, NJ], fp8, kind="ExternalInput").ap()
    csrow = nc.dram_tensor("csrow", [1, RPC], mybir.dt.float32,
                           kind="ExternalOutput").ap()
    rs = nc.dram_tensor("rs", [P, 32], mybir.dt.float32,
                        kind="ExternalOutput").ap()
    pose = nc.dram_tensor("pose", [P, NI], mybir.dt.float32,
                          kind="ExternalOutput").ap()
    with tile.TileContext(nc) as tc:
        with ExitStack() as ctx:
            _kernel_body(ctx, tc, csrow, rs, pose, zn, zt)
    return nc


_NC = None


def _get_nc(finalized=True):
    global _NC
    if _NC is None:
        _NC = build_nc()
    if finalized and not _NC.is_finalized():
        _NC.finalize()
    return _NC


def make_in_maps(z_orig, z_augment):
    from concourse import mybir

    f8np = mybir.dt.np(mybir.dt.float8e4)
    bfnp = mybir.dt.np(mybir.dt.bfloat16)
    z = np.ascontiguousarray(
        np.concatenate([np.asarray(z_augment, dtype=np.float32),
                        np.asarray(z_orig, dtype=np.float32)], axis=0))
    maps = []
    for c in range(N_CORES):
        zr = np.roll(z, -RPC * c, axis=0)[:NJ]
        zf8 = zr.astype(f8np)
        zbf = zr.astype(bfnp)
        # natural, pre-swizzled: zn[p, t*256 + c] = z[t*128+p, c]
        znat = np.ascontiguousarray(
            zbf.reshape(NT, P, D).transpose(1, 0, 2).reshape(P, NT * D))
        # transposed: zt[h, p, j] = z[j, 128h + p]
        zt = np.ascontiguousarray(zf8.T.reshape(2, P, NJ))
        maps.append({"zn": znat, "zt": zt})
    return maps


def reduce_outputs(results):
    """Host assembly: denominators from colsum + rowsums, ln, positives."""
    r = np.zeros(N, dtype=np.float64)
    pos_total = 0.0
    for c, res in enumerate(results):
        base = RPC * c
        cs = np.asarray(res["csrow"], dtype=np.float64).reshape(RPC)
        idx = (base + np.arange(RPC)) % N
        r[idx] += cs
        rsv = np.asarray(res["rs"], dtype=np.float64)  # [128, 32]
        for t in range(8, NT):
            jdx = (base + P * t + np.arange(P)) % N
            r[jdx] += rsv[:, t - 8]
        # positives: decode the exp-tile values back to 2 S_cos
        pe = np.asarray(res["pose"], dtype=np.float32)  # [128, 8]
        for k in range(NI):
            t = 32 + k
            v = pe[:, k].astype(np.float64)
            if ASSIGN[t] == 'A':
                # v = fp8(exp(2S - ln2)) -> 2S = ln(2 v)
                twos = np.log(np.maximum(2.0 * v, 1e-30))
            else:
                # v = fp8-bit-pattern Schraudolph: bits = 2S*8/ln2 + B - 8
                from concourse import mybir
                f8np = mybir.dt.np(mybir.dt.float8e4)
                bits = pe[:, k].astype(f8np).view(np.uint8).astype(np.float64)
                twos = (bits - (B_EXP - 8.0)) * LN2 / 8.0
            pos_total += float(twos.sum())
    total = float(np.log(np.maximum(r - E2, 1e-300)).sum()) - pos_total
    return np.float32(total / N)


def kernel(z_orig, z_augment):
    from concourse.bass_utils import run_bass_kernel_spmd

    nc = _get_nc()
    in_maps = make_in_maps(z_orig, z_augment)
    res = run_bass_kernel_spmd(nc, in_maps, core_ids=list(range(N_CORES)))
    return reduce_outputs(res.results)
